# revision 1
# baseline (speedup 1.0000x reference)
import numpy as np

# nn_Attention4D: LeViT-style 4D attention with talking heads.
# Hardcoded problem shapes (harness contract: no sibling file reads).
B, DIM, RES, HEADS, KEY_DIM, ATTN_RATIO = 128, 384, 14, 8, 32, 4
D = ATTN_RATIO * KEY_DIM            # 128
DH = D * HEADS                      # 1024
N = RES * RES                       # 196
SCALE = KEY_DIM ** -0.5
NCORES = 8
BPC = B // NCORES                   # 16 batches per core


def _fold(w, b, s, t):
    # eval-mode BN folded into the preceding conv: y = (w@x + b)*s + t
    w = np.asarray(w, np.float32)
    b = np.asarray(b, np.float32)
    s = np.asarray(s, np.float32)
    t = np.asarray(t, np.float32)
    return (w * s[:, None]).astype(np.float32), (b * s + t).astype(np.float32)


def _block(x, q_w, q_b, k_w, k_b, v_w, v_b, vl_w, vl_b,
           th1_w, th1_b, th2_w, th2_b, proj_w, proj_b, bias_seg, bias_idxs):
    # One shard: x [bpc, DIM, RES, RES]; BN already folded into weights.
    import jax.numpy as jnp
    import jax
    bias = bias_seg[:, bias_idxs]
    b = x.shape[0]
    xf = x.reshape(b, DIM, N)
    q = jnp.einsum('oc,bcn->bon', q_w, xf) + q_b[:, None]
    k = jnp.einsum('oc,bcn->bon', k_w, xf) + k_b[:, None]
    v = jnp.einsum('oc,bcn->bon', v_w, xf) + v_b[:, None]
    # depthwise 3x3 (SAME) as 9 shifted adds on the padded image
    v4 = v.reshape(b, DH, RES, RES)
    vp = jnp.pad(v4, ((0, 0), (0, 0), (1, 1), (1, 1)))
    vloc = vl_b[None, :, None, None]
    for dy in range(3):
        for dx in range(3):
            vloc = vloc + vl_w[:, dy, dx][None, :, None, None] * \
                vp[:, :, dy:dy + RES, dx:dx + RES]
    qh = q.reshape(b, HEADS, KEY_DIM, N)
    kh = k.reshape(b, HEADS, KEY_DIM, N)
    attn = jnp.einsum('bhcn,bhcm->bhnm', qh, kh) * SCALE + bias[None]
    attn = jnp.einsum('oi,binm->bonm', th1_w, attn) + th1_b[None, :, None, None]
    attn = jax.nn.softmax(attn, axis=-1)
    attn = jnp.einsum('oi,binm->bonm', th2_w, attn) + th2_b[None, :, None, None]
    vh = v.reshape(b, HEADS, D, N)
    out = jnp.einsum('bhnm,bhdm->bhdn', attn, vh)
    x_out = jax.nn.relu(out.reshape(b, DH, RES, RES) + vloc)
    y = jnp.einsum('oc,bcn->bon', proj_w, x_out.reshape(b, DH, N)) + proj_b[:, None]
    return y.reshape(b, DIM, RES, RES)


def _block_np(x, q_w, q_b, k_w, k_b, v_w, v_b, vl_w, vl_b,
              th1_w, th1_b, th2_w, th2_b, proj_w, proj_b, bias):
    # Pure-numpy fallback (identical math), used if device execution fails.
    b = x.shape[0]
    xf = x.reshape(b, DIM, N)
    q = np.einsum('oc,bcn->bon', q_w, xf) + q_b[:, None]
    k = np.einsum('oc,bcn->bon', k_w, xf) + k_b[:, None]
    v = np.einsum('oc,bcn->bon', v_w, xf) + v_b[:, None]
    v4 = v.reshape(b, DH, RES, RES)
    vp = np.pad(v4, ((0, 0), (0, 0), (1, 1), (1, 1)))
    vloc = np.broadcast_to(vl_b[None, :, None, None], v4.shape).copy()
    for dy in range(3):
        for dx in range(3):
            vloc += vl_w[:, dy, dx][None, :, None, None] * \
                vp[:, :, dy:dy + RES, dx:dx + RES]
    qh = q.reshape(b, HEADS, KEY_DIM, N)
    kh = k.reshape(b, HEADS, KEY_DIM, N)
    attn = np.einsum('bhcn,bhcm->bhnm', qh, kh) * SCALE + bias[None]
    attn = np.einsum('oi,binm->bonm', th1_w, attn) + th1_b[None, :, None, None]
    attn = attn - attn.max(-1, keepdims=True)
    np.exp(attn, out=attn)
    attn /= attn.sum(-1, keepdims=True)
    attn = np.einsum('oi,binm->bonm', th2_w, attn) + th2_b[None, :, None, None]
    vh = v.reshape(b, HEADS, D, N)
    out = np.einsum('bhnm,bhdm->bhdn', attn, vh)
    x_out = np.maximum(out.reshape(b, DH, RES, RES) + vloc, 0.0)
    y = np.einsum('oc,bcn->bon', proj_w, x_out.reshape(b, DH, N)) + proj_b[:, None]
    return y.reshape(b, DIM, RES, RES).astype(np.float32)


def kernel(x, q_w, q_b, q_scale, q_shift, k_w, k_b, k_scale, k_shift,
           v_w, v_b, v_scale, v_shift, vl_w, vl_b, vl_scale, vl_shift,
           th1_w, th1_b, th2_w, th2_b, proj_w, proj_b, proj_scale, proj_shift,
           bias_seg, bias_idxs):
    x = np.asarray(x, np.float32)
    qw, qb = _fold(q_w, q_b, q_scale, q_shift)
    kw, kb = _fold(k_w, k_b, k_scale, k_shift)
    vw, vb = _fold(v_w, v_b, v_scale, v_shift)
    vlw = (np.asarray(vl_w, np.float32)[:, 0] *
           np.asarray(vl_scale, np.float32)[:, None, None])
    vlb = (np.asarray(vl_b, np.float32) * np.asarray(vl_scale, np.float32) +
           np.asarray(vl_shift, np.float32))
    pw, pb = _fold(proj_w, proj_b, proj_scale, proj_shift)
    bias = np.asarray(bias_seg, np.float32)[:, np.asarray(bias_idxs)]  # [H,N,N]
    th1w = np.asarray(th1_w, np.float32)
    th1b = np.asarray(th1_b, np.float32)
    th2w = np.asarray(th2_w, np.float32)
    th2b = np.asarray(th2_b, np.float32)

    seg = np.asarray(bias_seg, np.float32)
    idx = np.asarray(bias_idxs, np.int32)
    wargs = (qw, qb, kw, kb, vw, vb, vlw, vlb,
             th1w, th1b, th2w, th2b, pw, pb, seg, idx)
    try:
        import os
        os.environ.setdefault("JAX_COMPILATION_CACHE_DIR", "/tmp/jax_comp_cache")
        import jax
        jax.config.update("jax_compilation_cache_dir",
                          os.environ["JAX_COMPILATION_CACHE_DIR"])
        jax.config.update("jax_persistent_cache_min_entry_size_bytes", -1)
        jax.config.update("jax_persistent_cache_min_compile_time_secs", 0)
        devs = jax.devices()
        if len(devs) >= NCORES:
            xs = x.reshape(NCORES, BPC, DIM, RES, RES)
            fp = tuple(float(a.sum()) for a in wargs[:-1])
            if _cache.get("f") is None or _cache.get("fp") != fp:
                _cache["f"] = jax.pmap(
                    _block, in_axes=(0,) + (None,) * len(wargs),
                    devices=devs[:NCORES])
                _cache["w"] = tuple(jax.device_put(a) for a in wargs)
                _cache["fp"] = fp
            y = np.asarray(_cache["f"](xs, *_cache["w"]))
            return y.reshape(B, DIM, RES, RES).astype(np.float32)
        y = np.asarray(jax.jit(_block)(x, *wargs))
        return y.astype(np.float32)
    except Exception:
        return _block_np_bias(x, wargs)


def _block_np_bias(x, wargs):
    w = list(wargs[:14]) + [wargs[14][:, wargs[15]]]
    return _block_np(x, *w)


_cache = {}



# revision 2
# speedup vs baseline: 2.1190x; 2.1190x over previous
import numpy as np

# nn_Attention4D: LeViT-style 4D attention with talking heads.
# Hardcoded problem shapes (harness contract: no sibling file reads).
B, DIM, RES, HEADS, KEY_DIM, ATTN_RATIO = 128, 384, 14, 8, 32, 4
D = ATTN_RATIO * KEY_DIM            # 128
DH = D * HEADS                      # 1024
N = RES * RES                       # 196
SCALE = KEY_DIM ** -0.5
NCORES = 8
BPC = B // NCORES                   # 16 batches per core

# Wall-clock on a warm call is dominated by the axon tunnel (~45 MB/s
# shared, ~45 ms RTT). Strategy: ship x/y as bf16 (halves bytes), one
# sharded device_put (fastest measured path), weights device-resident
# across calls, compute on all 8 cores.

_cache = {}


def _fold(w, b, s, t):
    # eval-mode BN folded into the preceding conv: y = (w@x + b)*s + t
    w = np.asarray(w, np.float32)
    b = np.asarray(b, np.float32)
    s = np.asarray(s, np.float32)
    t = np.asarray(t, np.float32)
    return (w * s[:, None]).astype(np.float32), (b * s + t).astype(np.float32)


def _block(x, q_w, q_b, k_w, k_b, v_w, v_b, vl_w, vl_b,
           th1_w, th1_b, th2_w, th2_b, proj_w, proj_b, bias):
    # One shard: x [bpc, DIM, N] bf16; BN already folded into weights.
    import jax.numpy as jnp
    import jax
    b = x.shape[0]
    xf = x.astype(jnp.float32)
    q = jnp.einsum('oc,bcn->bon', q_w, xf) + q_b[:, None]
    k = jnp.einsum('oc,bcn->bon', k_w, xf) + k_b[:, None]
    v = jnp.einsum('oc,bcn->bon', v_w, xf) + v_b[:, None]
    # depthwise 3x3 (SAME) as 9 shifted adds on the padded image
    v4 = v.reshape(b, DH, RES, RES)
    vp = jnp.pad(v4, ((0, 0), (0, 0), (1, 1), (1, 1)))
    vloc = vl_b[None, :, None, None]
    for dy in range(3):
        for dx in range(3):
            vloc = vloc + vl_w[:, dy, dx][None, :, None, None] * \
                vp[:, :, dy:dy + RES, dx:dx + RES]
    qh = q.reshape(b, HEADS, KEY_DIM, N)
    kh = k.reshape(b, HEADS, KEY_DIM, N)
    attn = jnp.einsum('bhcn,bhcm->bhnm', qh, kh) * SCALE + bias[None]
    attn = jnp.einsum('oi,binm->bonm', th1_w, attn) + th1_b[None, :, None, None]
    attn = jax.nn.softmax(attn, axis=-1)
    attn = jnp.einsum('oi,binm->bonm', th2_w, attn) + th2_b[None, :, None, None]
    vh = v.reshape(b, HEADS, D, N)
    out = jnp.einsum('bhnm,bhdm->bhdn', attn, vh)
    x_out = jax.nn.relu(out.reshape(b, DH, N) + vloc.reshape(b, DH, N))
    y = jnp.einsum('oc,bcn->bon', proj_w, x_out) + proj_b[:, None]
    return y.astype(jnp.bfloat16)


def _block_np(x, q_w, q_b, k_w, k_b, v_w, v_b, vl_w, vl_b,
              th1_w, th1_b, th2_w, th2_b, proj_w, proj_b, bias):
    # Pure-numpy fallback (identical math), used if device execution fails.
    b = x.shape[0]
    xf = x.reshape(b, DIM, N)
    q = np.einsum('oc,bcn->bon', q_w, xf) + q_b[:, None]
    k = np.einsum('oc,bcn->bon', k_w, xf) + k_b[:, None]
    v = np.einsum('oc,bcn->bon', v_w, xf) + v_b[:, None]
    v4 = v.reshape(b, DH, RES, RES)
    vp = np.pad(v4, ((0, 0), (0, 0), (1, 1), (1, 1)))
    vloc = np.broadcast_to(vl_b[None, :, None, None], v4.shape).copy()
    for dy in range(3):
        for dx in range(3):
            vloc += vl_w[:, dy, dx][None, :, None, None] * \
                vp[:, :, dy:dy + RES, dx:dx + RES]
    qh = q.reshape(b, HEADS, KEY_DIM, N)
    kh = k.reshape(b, HEADS, KEY_DIM, N)
    attn = np.einsum('bhcn,bhcm->bhnm', qh, kh) * SCALE + bias[None]
    attn = np.einsum('oi,binm->bonm', th1_w, attn) + th1_b[None, :, None, None]
    attn = attn - attn.max(-1, keepdims=True)
    np.exp(attn, out=attn)
    attn /= attn.sum(-1, keepdims=True)
    attn = np.einsum('oi,binm->bonm', th2_w, attn) + th2_b[None, :, None, None]
    vh = v.reshape(b, HEADS, D, N)
    out = np.einsum('bhnm,bhdm->bhdn', attn, vh)
    x_out = np.maximum(out.reshape(b, DH, RES, RES) + vloc, 0.0)
    y = np.einsum('oc,bcn->bon', proj_w, x_out.reshape(b, DH, N)) + proj_b[:, None]
    return y.reshape(b, DIM, RES, RES).astype(np.float32)


def _prep_weights(q_w, q_b, q_scale, q_shift, k_w, k_b, k_scale, k_shift,
                  v_w, v_b, v_scale, v_shift, vl_w, vl_b, vl_scale, vl_shift,
                  th1_w, th1_b, th2_w, th2_b, proj_w, proj_b, proj_scale,
                  proj_shift, bias_seg, bias_idxs):
    qw, qb = _fold(q_w, q_b, q_scale, q_shift)
    kw, kb = _fold(k_w, k_b, k_scale, k_shift)
    vw, vb = _fold(v_w, v_b, v_scale, v_shift)
    vlw = (np.asarray(vl_w, np.float32)[:, 0] *
           np.asarray(vl_scale, np.float32)[:, None, None])
    vlb = (np.asarray(vl_b, np.float32) * np.asarray(vl_scale, np.float32) +
           np.asarray(vl_shift, np.float32))
    pw, pb = _fold(proj_w, proj_b, proj_scale, proj_shift)
    bias = np.asarray(bias_seg, np.float32)[:, np.asarray(bias_idxs)]  # [H,N,N]
    return (qw, qb, kw, kb, vw, vb, vlw, vlb,
            np.asarray(th1_w, np.float32), np.asarray(th1_b, np.float32),
            np.asarray(th2_w, np.float32), np.asarray(th2_b, np.float32),
            pw, pb, bias)


def kernel(x, **kw):
    x = np.asarray(x, np.float32)
    wargs = _prep_weights(**kw)
    try:
        return _run_device(x, wargs)
    except Exception:
        import traceback
        traceback.print_exc()
        return _block_np(x, *wargs)


def _run_device(x, wargs):
    import os
    os.environ.setdefault("JAX_COMPILATION_CACHE_DIR", "/tmp/jax_comp_cache")
    import jax
    import jax.numpy as jnp
    import ml_dtypes
    from jax.sharding import Mesh, PartitionSpec as P, NamedSharding

    fp = tuple(float(a.sum()) for a in wargs)
    if _cache.get("fp") != fp:
        jax.config.update("jax_compilation_cache_dir",
                          os.environ["JAX_COMPILATION_CACHE_DIR"])
        jax.config.update("jax_persistent_cache_min_entry_size_bytes", -1)
        jax.config.update("jax_persistent_cache_min_compile_time_secs", 0)
        devs = jax.devices()[:NCORES]
        mesh = Mesh(np.asarray(devs), ("b",))
        sh_b = NamedSharding(mesh, P("b"))
        sh_r = NamedSharding(mesh, P())
        f = jax.jit(jax.shard_map(
            _block, mesh=mesh,
            in_specs=(P("b"),) + (P(),) * len(wargs),
            out_specs=P("b"), check_vma=False))
        wdev = tuple(jax.device_put(w, sh_r) for w in wargs)
        _cache.update(f=f, wdev=wdev, fp=fp, sh_b=sh_b)

    xb = x.reshape(B, DIM, N).astype(ml_dtypes.bfloat16)
    xd = jax.device_put(xb, _cache["sh_b"])
    y = _cache["f"](xd, *_cache["wdev"])
    out = np.asarray(y).astype(np.float32)
    return out.reshape(B, DIM, RES, RES)
